# revision 22
# baseline (speedup 1.0000x reference)
"""Trainium2 Bass kernel: ResNet BasicBlock (conv3x3-BN-ReLU-mask-conv3x3-mask-BN-residual-ReLU).

Problem shape: x[4096, 64, 7, 7], both convs 64->64 3x3 pad 1.

Strategy (pure data parallel, 8 cores, 512 images/core):
  * Channels live on SBUF partitions. Two 64-channel image streams are
    stacked into the 128 partitions ("half0" -> partitions 0-63,
    "half1" -> 64-127) so elementwise engines run at full width.
  * A 3x3 conv is 9 shifted 64x64 matmuls accumulated in PSUM. Images are
    zero-padded to 9x9 on-chip; each tap reads a strided window of the
    padded tile. Matmul operands are bf16 (fp32 matmuls lower to two PE
    passes - 4x the cost); accumulation stays fp32 in PSUM.
  * The 128x128 PE array is split into 4 64x64 quadrants via the matmul
    base partitions (rhs base -> row group, psum base -> column group).
    Four independent tap-chains (2 pairs x 2 halves) run concurrently, so
    the array is fully utilized despite C=64.
  * BN scales are folded into the conv weights on the host; BN shifts are
    per-partition bias/scalar operands. The identity residual is folded
    into conv2 as a 10th tap with an identity weight matrix, so no
    separate elementwise add pass is needed.
  * The critic masks only touch batch element 0: every core runs the same
    mask multiply on its first image, but cores 1-7 get all-ones masks.

Layouts are precomputed on the host (numpy) so every DMA is a clean
contiguous 128-partition transfer.
"""

import ml_dtypes
import numpy as np

import concourse.bass as bass  # noqa: F401  (engine namespaces live on the nc object)
import concourse.tile as tile
from concourse import bacc, mybir
from concourse.bass_utils import run_bass_kernel_spmd

F32 = mybir.dt.float32
BF16 = mybir.dt.bfloat16
NP_BF16 = ml_dtypes.bfloat16
EPS = 1e-5
B, C, H, W = 4096, 64, 7, 7
NCORES = 8
BPC = B // NCORES          # 512 images per core
SLOTS = BPC // 2           # 256 image slots per half-stream
NPAIR = SLOTS // 8         # 32 pairs (a pair = 8 half0 + 8 half1 images)
NQUAD = NPAIR // 2         # 16 quads (a quad = 2 pairs = 4 matmul chains)
NGRP = 4                   # DMA groups per core
QPG = NQUAD // NGRP        # 4 quads per group
SPG = SLOTS // NGRP        # 64 slots per group
NIMG = 8                   # images per chain
NFREE = NIMG * H * W       # 392 = matmul free dim
NBUF = 3                   # padded-tile ring depth

# (pair_in_quad, half, colgroup): the 4 concurrent chains of a quad.
# Even pair writes PSUM naturally, odd pair swapped - this alternation is
# what keeps all four PE quadrants busy across consecutive chains. Order
# within a round: the two streams sharing an XBUS column-group adjacent.
CHAINS = [(0, 0, 0), (1, 1, 0), (0, 1, 1), (1, 0, 1)]

_CACHE = {}


def _psum_view(ps, j):
    """[128, 8, 7, 7] view of pair j's bank of a [128, 2, 512] psum tile."""
    return ps[:, j, 0:NFREE].rearrange("p (i h w) -> p i h w", i=NIMG, h=H, w=W)


def _emit_conv(nc, src_pad, w_sb, ps, has_identity=False):
    """One quad of one conv: 4 concurrent 9-tap chains (36 matmuls)."""
    last = 8 if not has_identity else 9
    for t in range(9):
        dh, dw = t // 3, t % 3
        for (j, half, cg) in CHAINS:
            rhs = src_pad[64 * half:64 * half + 64,
                          NIMG * j:NIMG * (j + 1), dh:dh + 7, dw:dw + 7]
            lhsT = w_sb[64 * half:64 * half + 64, t, :]
            out = ps[64 * cg:64 * cg + 64, j, 0:NFREE]
            nc.tensor.matmul(out, lhsT, rhs, start=(t == 0), stop=(t == last))


def _emit_identity_tap(nc, eye_sb, x_pad, ps):
    """10th conv2 tap: accumulate the residual x via an identity weight.

    The row group follows the half that actually holds the chain's images
    in the (always natural) x_pad tile.
    """
    for (j, half, cg) in CHAINS:
        dhalf = half if j == 0 else 1 - half
        rhs = x_pad[64 * dhalf:64 * dhalf + 64,
                    NIMG * j:NIMG * (j + 1), 1:8, 1:8]
        lhsT = eye_sb[64 * dhalf:64 * dhalf + 64, :]
        out = ps[64 * cg:64 * cg + 64, j, 0:NFREE]
        nc.tensor.matmul(out, lhsT, rhs, start=False, stop=True)


def _build():
    nc = bacc.Bacc("TRN2", target_bir_lowering=False, debug=False,
                   num_devices=NCORES)
    x_d = nc.dram_tensor("x", [128, SLOTS, H, W], F32, kind="ExternalInput")
    w1_d = nc.dram_tensor("w1", [128, 9, 64], BF16, kind="ExternalInput")
    w2_d = nc.dram_tensor("w2", [128, 9, 64], BF16, kind="ExternalInput")
    eye_d = nc.dram_tensor("eye", [128, 64], BF16, kind="ExternalInput")
    cst_d = nc.dram_tensor("cst", [128, 2], F32, kind="ExternalInput")
    msk_d = nc.dram_tensor("msk", [64, 2, H, W], F32, kind="ExternalInput")
    o_d = nc.dram_tensor("o", [128, SLOTS, H, W], F32, kind="ExternalOutput")

    with tile.TileContext(nc) as tc:
        with (
            tc.tile_pool(name="singles", bufs=1) as singles,
            tc.tile_pool(name="xin", bufs=3) as xin_pool,
            tc.tile_pool(name="outp", bufs=3) as out_pool,
            tc.tile_pool(name="pads", bufs=1) as pad_pool,
            tc.tile_pool(name="ps1", bufs=2, space="PSUM") as ps1_pool,
            tc.tile_pool(name="ps2", bufs=2, space="PSUM") as ps2_pool,
        ):
            w1_sb = singles.tile([128, 9, 64], BF16, name="w1_sb")
            w2_sb = singles.tile([128, 9, 64], BF16, name="w2_sb")
            eye_sb = singles.tile([128, 64], BF16, name="eye_sb")
            cst_sb = singles.tile([128, 2], F32, name="cst_sb")
            msk_sb = singles.tile([64, 2, H, W], F32, name="msk_sb")
            warm_sb = singles.tile([128, 1], F32, name="warm_sb")

            def emit_const_dmas():
                # SWDGE (gpsimd) queue so these don't delay the bulk x
                # loads on the HWDGE ring; ordered by first use.
                nc.gpsimd.dma_start(w1_sb[:], w1_d[:])
                nc.gpsimd.dma_start(cst_sb[:], cst_d[:])
                nc.gpsimd.dma_start(w2_sb[:], w2_d[:])
                nc.gpsimd.dma_start(eye_sb[:], eye_d[:])
                nc.gpsimd.dma_start(msk_sb[:], msk_d[:])

            # Preload the ACT function table (contains Relu) during the
            # DMA prologue instead of before the first real activation.
            nc.scalar.memzero(warm_sb[:])

            # Persistent zero-padded 9x9 image tiles: the border is zeroed
            # once and never rewritten (compute only touches the interior).
            # (memsets are emitted in emit_pad_init, after the first bulk
            # DMA and the const DMAs have been queued.)
            xpads, y1pads = [], []
            for i in range(NBUF):
                xp = pad_pool.tile([128, 2 * NIMG, 9, 9], BF16,
                                   name=f"xpad{i}", tag=f"xpad{i}")
                yp = pad_pool.tile([128, 2 * NIMG, 9, 9], BF16,
                                   name=f"y1pad{i}", tag=f"y1pad{i}")
                xpads.append(xp)
                y1pads.append(yp)

            def emit_pad_init():
                # spread across engines so no single queue serializes the
                # prologue; only xpad0 gates the first conv (xpad1/2 are
                # memset lazily in the main loop)
                nc.vector.memset(xpads[0][:], 0.0)
                # xpad2/3 are first used one superquad later; their memsets
                # are emitted lazily in the main loop (after the first two
                # pad copies) so they stay off the critical path.
                for yp in y1pads:
                    nc.gpsimd.memset(yp[:], 0.0)

            def emit_conv2(state):
                v, vq, g, xp, yp, out_g = state
                ps2 = ps2_pool.tile([128, 2, 512], F32, name="ps2t")
                _emit_conv(nc, yp, w2_sb, ps2, has_identity=True)
                if v == 0:
                    # critic mask 2 on conv2 output of batch element 0
                    # (before the identity-tap residual is accumulated)
                    tgt = ps2[0:64, 0, 0:H * W].rearrange(
                        "p (h w) -> p h w", h=H, w=W)
                    nc.vector.tensor_mul(tgt, tgt, msk_sb[:, 1, :, :])
                _emit_identity_tap(nc, eye_sb, xp, ps2)
                for j in range(2):
                    s0 = 16 * vq + NIMG * j  # slot offset within group
                    # out = relu(psum + shift2); psum already holds conv2+x.
                    # Split between DVE and ACT to balance engine load.
                    if j == 0:
                        nc.vector.tensor_scalar(
                            out_g[:, s0:s0 + NIMG], _psum_view(ps2, j),
                            cst_sb[:, 1:2], 0.0,
                            mybir.AluOpType.add, mybir.AluOpType.max)
                    else:
                        nc.scalar.activation(
                            out=out_g[:, s0:s0 + NIMG], in_=_psum_view(ps2, j),
                            func=mybir.ActivationFunctionType.Relu,
                            bias=cst_sb[:, 1:2], scale=1.0)
                # stream the output back in 400 KB per-quad chunks
                lo = 16 * vq
                nc.sync.dma_start(o_d[:, g * SPG + lo:g * SPG + lo + 16],
                                  out_g[:, lo:lo + 16])

            pending = None
            for g in range(NGRP):
                xin_g = xin_pool.tile([128, SPG, H, W], F32, name="xin_g")
                if g == 0:
                    # first pair's data in a small DMA so compute starts
                    # sooner; constants ride the gpsimd queue behind it
                    nc.sync.dma_start(xin_g[:, 0:8], x_d[:, 0:8])
                    emit_const_dmas()
                    nc.sync.dma_start(xin_g[:, 8:16], x_d[:, 8:16])
                    emit_pad_init()
                    nc.sync.dma_start(xin_g[:, 16:32], x_d[:, 16:32])
                    nc.sync.dma_start(xin_g[:, 32:SPG], x_d[:, 32:SPG])
                else:
                    nc.sync.dma_start(xin_g[:, 0:SPG // 2],
                                      x_d[:, g * SPG:g * SPG + SPG // 2])
                    nc.sync.dma_start(xin_g[:, SPG // 2:SPG],
                                      x_d[:, g * SPG + SPG // 2:(g + 1) * SPG])
                out_g = out_pool.tile([128, SPG, H, W], F32, name="out_g")
                for vq in range(QPG):
                    v = g * QPG + vq
                    xp = xpads[v % NBUF]
                    if 1 <= v < NBUF:
                        # lazy border memset for the ring slots not covered
                        # in emit_pad_init (keeps the prologue short)
                        nc.vector.memset(xp[:], 0.0)
                    if v == 0:
                        # split so the j=0 chains' matmuls only wait on the
                        # first 8-slot DMA chunk
                        nc.vector.tensor_copy(
                            xp[:, 0:NIMG, 1:8, 1:8], xin_g[:, 0:NIMG])
                        nc.vector.tensor_copy(
                            xp[:, NIMG:2 * NIMG, 1:8, 1:8],
                            xin_g[:, NIMG:2 * NIMG])
                    else:
                        nc.vector.tensor_copy(
                            xp[:, :, 1:8, 1:8], xin_g[:, 16 * vq:16 * vq + 16])
                    ps1 = ps1_pool.tile([128, 2, 512], F32, name="ps1t")
                    _emit_conv(nc, xp, w1_sb, ps1)
                    yp = y1pads[v % NBUF]
                    for j in range(2):
                        nc.scalar.activation(
                            out=yp[:, NIMG * j:NIMG * (j + 1), 1:8, 1:8],
                            in_=_psum_view(ps1, j),
                            func=mybir.ActivationFunctionType.Relu,
                            bias=cst_sb[:, 0:1], scale=1.0)
                    if v == 0:
                        # critic mask 1 on relu(bn1(conv1)) of batch elem 0
                        tgt = yp[0:64, 0, 1:8, 1:8]
                        nc.vector.tensor_mul(tgt, tgt, msk_sb[:, 0, :, :])
                    if pending is not None:
                        emit_conv2(pending)
                    pending = (v, vq, g, xp, yp, out_g)
            emit_conv2(pending)

    nc.compile()
    return nc


def _get_nc():
    if "nc" not in _CACHE:
        _CACHE["nc"] = _build()
    return _CACHE["nc"]


def _host_pack(x, w1, g1, b1, m1, v1, w2, g2, b2, m2, v2, mask1, mask2):
    x = np.ascontiguousarray(np.asarray(x, np.float32))
    scale1 = np.asarray(g1, np.float32) / np.sqrt(np.asarray(v1, np.float32) + EPS)
    shift1 = np.asarray(b1, np.float32) - np.asarray(m1, np.float32) * scale1
    scale2 = np.asarray(g2, np.float32) / np.sqrt(np.asarray(v2, np.float32) + EPS)
    shift2 = np.asarray(b2, np.float32) - np.asarray(m2, np.float32) * scale2

    def pack_w(w, scale):
        ws = np.asarray(w, np.float32) * scale[:, None, None, None]
        # [co, ci, kh, kw] -> [ci, tap, co], duplicated into both halves
        lhsT = ws.transpose(1, 2, 3, 0).reshape(64, 9, 64)
        return np.ascontiguousarray(np.tile(lhsT, (2, 1, 1)).astype(NP_BF16))

    wdev1, wdev2 = pack_w(w1, scale1), pack_w(w2, scale2)
    eye = np.ascontiguousarray(np.tile(np.eye(64), (2, 1)).astype(NP_BF16))
    cst = np.tile(np.stack([shift1, shift2], 1), (2, 1))
    cst = np.ascontiguousarray(cst.astype(np.float32))

    # [core, pair, half, img, ch, h, w] -> [core, half*ch, pair*img, h, w]
    xr = x.reshape(NCORES, NPAIR, 2, NIMG, C, H, W)
    xdev = np.ascontiguousarray(
        xr.transpose(0, 2, 4, 1, 3, 5, 6).reshape(NCORES, 128, SLOTS, H, W))

    msk0 = np.ascontiguousarray(
        np.stack([np.asarray(mask1, np.float32),
                  np.asarray(mask2, np.float32)], 1))
    msk1s = np.ones_like(msk0)

    in_maps = []
    for c in range(NCORES):
        in_maps.append({
            "x": xdev[c],
            "w1": wdev1,
            "w2": wdev2,
            "eye": eye,
            "cst": cst,
            "msk": msk0 if c == 0 else msk1s,
        })
    return in_maps


def _host_unpack(results):
    o = np.stack([results[c]["o"] for c in range(NCORES)])
    o = o.reshape(NCORES, 2, C, NPAIR, NIMG, H, W)
    return np.ascontiguousarray(
        o.transpose(0, 3, 1, 4, 2, 5, 6).reshape(B, C, H, W))


def run(trace=False, **inputs):
    nc = _get_nc()
    in_maps = _host_pack(**inputs)
    res = run_bass_kernel_spmd(nc, in_maps, core_ids=list(range(NCORES)),
                               trace=trace)
    return _host_unpack(res.results), res


def kernel(**inputs) -> np.ndarray:
    out, _ = run(trace=False, **inputs)
    return out
